# revision 17
# baseline (speedup 1.0000x reference)
"""Trainium2 Bass kernel for 16-head MHA (B=2, S=2048, D=1024), fp32 I/O.

Sharding: 2-way batch-parallel x 4-way head-parallel across 8 NeuronCores.
Core c = (b, g) owns batch b and head group g (4 heads, a 256-wide slice of
the QKV projection output and of Wo's input dim). Each core computes its
(batch, head-group)'s full attention plus a partial output projection over
its batch's tokens; the host sums the 4 partials per batch. Versus pure
head-sharding this halves both input DMA (12MB/core) and output DMA (8MB).

Per-core dataflow (feature-major; host pre-transposes and casts to bf16):
  QT/KT [2x 128, 2048] bf16 = W_pair @ x.T    per head-pair tile
  V+ tiles [128j, 65] bf16                    VT j-chunks PE-transposed + ones
  windows = (i-block 512, head-pair t), t-major order, 8 windows:
    S.T pair [128j, 1024] = KT.T @ QT         both heads of the pair land in
                                              opposite PE row-halves and run
                                              CONCURRENTLY (row tiling)
    E = exp(S.T/8)                            one ACT instr per pair
    O+ [65, 512] += V+.T @ E_h                ones row = softmax denominator
    oc = O+[0:64] * replicate(1/O+[64])       DVE + PE outer-product replicate
    out rows = oc.T @ WoT  (fp32r)            partial projection
  K/V/Q projections and Wo are interleaved into the window loop as
  background work so the PE never idles; exp is the only ACT work and all
  DMA dispatch lives on SP (inputs, deadline-ordered) and DVE (outputs).

PSUM budget (8 banks): scores pair tiles 2x2 + AV accumulators 2 + shared
aux (proj/Wo/transpose/replicate) 2.
"""

import sys

sys.path.insert(0, "/opt/trn_rl_repo")

import numpy as np

import concourse.bacc as bacc
import concourse.mybir as mybir
import concourse.tile as tile
from concourse.bass_utils import run_bass_kernel_spmd
from concourse.masks import make_identity

F32 = mybir.dt.float32
R = mybir.dt.float32r
BF16 = mybir.dt.bfloat16
EXP = mybir.ActivationFunctionType.Exp

D = 1024
BATCH = 2
SEQ = 2048
DK = 64
NH = 4  # heads per core
HG = NH * DK  # 256-wide head-group slice per core
NP = 2  # head-pairs per core
N_CORES = 8
KT_TILES = D // 128  # 8 contraction tiles for the projections
NCH = SEQ // 512  # 4 token chunks
JC = SEQ // 128  # 16 j-chunks
IB = 512  # i-block (query window)
NIB = SEQ // IB  # 4
SCALE = 1.0 / np.sqrt(DK)


def build_bass():
    nc = bacc.Bacc(None)

    xq = nc.dram_tensor("xq", [D, SEQ], BF16, kind="ExternalInput")
    xk = nc.dram_tensor("xk", [D, SEQ], BF16, kind="ExternalInput")
    xv = nc.dram_tensor("xv", [D, SEQ], BF16, kind="ExternalInput")
    wq = nc.dram_tensor("wq", [D, HG], BF16, kind="ExternalInput")
    wk = nc.dram_tensor("wk", [D, HG], BF16, kind="ExternalInput")
    wv = nc.dram_tensor("wv", [D, HG], BF16, kind="ExternalInput")
    wo = nc.dram_tensor("wo", [HG, D], R, kind="ExternalInput")
    out = nc.dram_tensor("out", [SEQ, D], F32, kind="ExternalOutput")

    with tile.TileContext(nc) as tc:
        with (
            tc.tile_pool(name="consts", bufs=1) as cst,
            tc.tile_pool(name="wpool", bufs=1) as wpool,
            tc.tile_pool(name="acts", bufs=1) as acts,
            tc.tile_pool(name="vp", bufs=1) as vp_pool,
            tc.tile_pool(name="stage", bufs=1) as stage,
            tc.tile_pool(name="vtc", bufs=2) as vtc,
            tc.tile_pool(name="epool", bufs=20) as epool,
            tc.tile_pool(name="ocpool", bufs=1) as ocpool,
            tc.tile_pool(name="outpool", bufs=2) as outpool,
            tc.tile_pool(name="small", bufs=2) as small,
            tc.tile_pool(name="psb", bufs=2, space="PSUM") as psb,
            tc.tile_pool(name="pso", bufs=2, space="PSUM") as pso,
            tc.tile_pool(name="paux", bufs=2, space="PSUM") as paux,
        ):
            # --- constants ---
            ident_f = cst.tile([128, 128], F32)
            make_identity(nc, ident_f)
            ident = cst.tile([128, 128], R)
            nc.vector.tensor_copy(ident[:], ident_f[:])

            ones_f = cst.tile([128, 1], F32)
            nc.gpsimd.memset(ones_f[:], 1.0)
            onescol = cst.tile([128, 1], BF16)
            nc.vector.tensor_copy(onescol[:], ones_f[:])
            ones64 = cst.tile([1, 64], R)
            nc.vector.tensor_copy(ones64[:], ones_f[0:1, 0:1].to_broadcast([1, 64]))

            # warm the ACT exp table while DMA streams inputs
            scratch = cst.tile([1, 64], F32)
            nc.scalar.activation(
                scratch[:], ones_f[0:1, 0:1].to_broadcast([1, 64]), EXP
            )

            # --- weight tiles ---
            wq_sb = wpool.tile([128, KT_TILES, HG], BF16)
            wk_sb = wpool.tile([128, KT_TILES, HG], BF16)
            wv_sb = wpool.tile([128, KT_TILES, HG], BF16)
            wo_sb = [wpool.tile([128, D], R, name=f"wo{t}") for t in range(NP)]

            # --- activation tiles ---
            QT = [acts.tile([128, SEQ], BF16, name=f"QT{t}") for t in range(NP)]
            KT = [acts.tile([128, SEQ], BF16, name=f"KT{t}") for t in range(NP)]

            # --- input staging (chunks held until their pair-1 proj reads) ---
            xst = {}  # (tensor_key, chunk) -> tile
            x_r = {
                "q": xq.rearrange("(ko p) n -> p ko n", p=128),
                "k": xk.rearrange("(ko p) n -> p ko n", p=128),
                "v": xv.rearrange("(ko p) n -> p ko n", p=128),
            }

            def dma_chunk(key, c, bufs, eng=None):
                t = stage.tile(
                    [128, KT_TILES, 512], BF16, tag=f"x{key}", bufs=bufs,
                    name=f"x{key}{c}",
                )
                (eng or nc.sync).dma_start(
                    t[:], x_r[key][:, :, c * 512 : (c + 1) * 512]
                )
                xst[(key, c)] = t

            # Input DMA: K/Q stream on the SP HWDGE ring, V/Wo on the ACT
            # HWDGE ring (dispatched before the first exp, transfers run
            # concurrently with the K stream so window 0's K chunks are not
            # queued behind V). k/q chunks are consumed by both head-pairs'
            # projections adjacently (bufs=2); v chunks stay staged until the
            # deferred pair-1 V projection (bufs=4).
            nc.sync.dma_start(wk_sb[:], wk.rearrange("(ko p) n -> p ko n", p=128))
            dma_chunk("k", 0, 2)
            nc.sync.dma_start(wq_sb[:], wq.rearrange("(ko p) n -> p ko n", p=128))
            dma_chunk("q", 0, 2)
            nc.scalar.dma_start(
                wv_sb[:], wv.rearrange("(ko p) n -> p ko n", p=128)
            )
            dma_chunk("k", 1, 2)
            dma_chunk("v", 0, 4, eng=nc.scalar)
            dma_chunk("k", 2, 2)
            dma_chunk("v", 1, 4, eng=nc.scalar)
            dma_chunk("k", 3, 2)
            dma_chunk("v", 2, 4, eng=nc.scalar)
            dma_chunk("q", 1, 2)
            dma_chunk("v", 3, 4, eng=nc.scalar)
            dma_chunk("q", 2, 2)
            dma_chunk("q", 3, 2)
            for t in range(NP):
                nc.scalar.dma_start(wo_sb[t][:], wo[t * 128 : (t + 1) * 128, :])

            w_sb = {"q": wq_sb, "k": wk_sb, "v": wv_sb}
            vp_tiles = {}  # (h, jg) -> V+ tile
            vt_chunks = {}  # (t, c) -> VT chunk tile (fp32r)

            def proj_group(key, c, t):
                """Project x[key] chunk c for head-pair t."""
                pp = paux.tile([128, 512], F32, tag="aux", name=f"pp_{key}{c}{t}")
                for k in range(KT_TILES):
                    nc.tensor.matmul(
                        pp[:],
                        w_sb[key][:, k, t * 128 : (t + 1) * 128],
                        xst[(key, c)][:, k, :],
                        start=(k == 0),
                        stop=(k == KT_TILES - 1),
                    )
                if key == "v":
                    vt = vtc.tile([128, 512], R, tag="vt", name=f"vt{t}{c}")
                    nc.vector.tensor_copy(vt[:], pp[:])
                    vt_chunks[(t, c)] = vt
                else:
                    TT = QT[t] if key == "q" else KT[t]
                    nc.vector.tensor_copy(TT[:, c * 512 : (c + 1) * 512], pp[:])

            def vplus(t, jgs):
                """Build V+ tiles for both heads of pair t, j-groups jgs."""
                for jg in jgs:
                    vt = vt_chunks[(t, jg // 4)]
                    jl = (jg % 4) * 128
                    for r in range(2):
                        h = 2 * t + r
                        hs = slice(r * 64, (r + 1) * 64)
                        tp = paux.tile([128, 64], R, tag="aux", name=f"tp{h}{jg}")
                        nc.tensor.transpose(
                            tp[:], vt[hs, jl : jl + 128], ident[hs, hs]
                        )
                        vpt = vp_pool.tile(
                            [128, DK + 1], BF16, tag=f"vp_{h}_{jg}", name=f"vp{h}{jg}"
                        )
                        nc.vector.tensor_copy(vpt[:, 0:DK], tp[:])
                        nc.vector.tensor_copy(vpt[:, DK : DK + 1], onescol[:])
                        vp_tiles[(h, jg)] = vpt

            ocs = {}  # (c, t) -> oc tile [128, 512] fp32r

            def emit_normalize(st):
                c, t, po = st["c"], st["t"], st["po"]
                oc = ocpool.tile([128, IB], R, tag=f"oc{t}", bufs=2, name=f"oc{c}{t}")
                # copy the AV accumulators out of PSUM first so the po banks
                # free up for the next window's AV almost immediately
                posb = {}
                for r in range(2):
                    posb[r] = small.tile(
                        [DK + 1, IB], F32, tag="posb", bufs=4, name=f"pb{r}"
                    )
                    nc.vector.tensor_copy(posb[r][:], po[r][:])
                for r in range(2):
                    rrow = small.tile([1, IB], F32, tag="rrow", name=f"rr{r}")
                    nc.vector.reciprocal(rrow[:], posb[r][DK : DK + 1, :])
                    rcr = small.tile([1, IB], R, tag="rcr", name=f"rcr{r}")
                    nc.vector.tensor_copy(rcr[:], rrow[:])
                    rep = paux.tile([64, IB], F32, tag="aux", name=f"rep{r}")
                    nc.tensor.matmul(rep[:], ones64[:], rcr[:], start=True, stop=True)
                    rec_sb = small.tile([64, IB], F32, tag="recsb", name=f"rs{r}")
                    nc.vector.tensor_copy(rec_sb[:], rep[:])
                    nc.vector.tensor_tensor(
                        oc[r * 64 : (r + 1) * 64, :],
                        posb[r][0:DK, :],
                        rec_sb[:],
                        mybir.AluOpType.mult,
                    )
                ocs[(c, t)] = oc

            def wo_piece(c, tt):
                """Output projection for token tile tt of i-block c."""
                os_t = outpool.tile([128, D], F32, tag="os", name=f"os{c}{tt}")
                for oh in range(2):
                    wo_ps = paux.tile(
                        [128, 512], F32, tag="aux", name=f"wops{c}{tt}{oh}"
                    )
                    for t in range(NP):
                        nc.tensor.matmul(
                            wo_ps[:],
                            ocs[(c, t)][:, tt * 128 : (tt + 1) * 128],
                            wo_sb[t][:, oh * 512 : (oh + 1) * 512],
                            start=(t == 0),
                            stop=(t == NP - 1),
                        )
                    nc.vector.tensor_copy(
                        os_t[:, oh * 512 : (oh + 1) * 512], wo_ps[:]
                    )
                r0 = c * IB + tt * 128
                nc.sync.dma_start(out[r0 : r0 + 128, :], os_t[:])

            def emit_scores(st, jc):
                c, t = st["c"], st["t"]
                i0 = c * IB
                ps = psb.tile([128, 2 * IB], F32, tag="ps", name=f"ps{jc}")
                for r in range(2):
                    hs = slice(r * 64, (r + 1) * 64)
                    nc.tensor.matmul(
                        ps[:, r * IB : (r + 1) * IB],
                        KT[t][hs, jc * 128 : (jc + 1) * 128],
                        QT[t][hs, i0 : i0 + IB],
                        start=True,
                        stop=True,
                    )
                e_t = epool.tile([128, 2 * IB], BF16, tag="e", name=f"e{jc}")
                nc.scalar.activation(e_t[:], ps[:], EXP, scale=SCALE)
                st["e"][jc] = e_t

            def emit_av(st, jc):
                t = st["t"]
                if jc == 0:
                    st["po"] = {
                        r: pso.tile([DK + 1, IB], F32, tag="po", name=f"po{r}")
                        for r in range(2)
                    }
                po = st["po"]
                for r in range(2):
                    nc.tensor.matmul(
                        po[r][:],
                        vp_tiles[(2 * t + r, jc)][:],
                        st["e"][jc][:, r * IB : (r + 1) * IB],
                        start=(jc == 0),
                        stop=(jc == JC - 1),
                    )

            # --- prefix: chunk-0 K/Q for both pairs ---
            proj_group("k", 0, 0)
            proj_group("k", 0, 1)
            proj_group("q", 0, 0)
            proj_group("q", 0, 1)

            # --- background work plan (c-major window order) ---
            bg = {
                0: [
                    lambda: proj_group("k", 1, 0),
                    lambda: proj_group("k", 1, 1),
                    lambda: proj_group("k", 2, 0),
                    lambda: proj_group("k", 2, 1),
                    lambda: proj_group("k", 3, 0),
                    lambda: proj_group("k", 3, 1),
                    lambda: proj_group("v", 0, 0),
                    lambda: vplus(0, [0, 1]),
                    lambda: vplus(0, [2, 3]),
                    lambda: proj_group("v", 1, 0),
                    lambda: vplus(0, [4, 5]),
                ],
                1: [
                    lambda: vplus(0, [6, 7]),
                    lambda: proj_group("v", 2, 0),
                    lambda: vplus(0, [8, 9]),
                    lambda: vplus(0, [10, 11]),
                    lambda: proj_group("v", 3, 0),
                    lambda: vplus(0, [12, 13]),
                    lambda: vplus(0, [14, 15]),
                    lambda: proj_group("v", 0, 1),
                    lambda: vplus(1, [0, 1]),
                    lambda: vplus(1, [2, 3]),
                    lambda: proj_group("q", 1, 0),
                    lambda: proj_group("q", 1, 1),
                ],
                2: [
                    lambda: proj_group("v", 1, 1),
                    lambda: vplus(1, [4, 5]),
                    lambda: vplus(1, [6, 7]),
                    lambda: proj_group("v", 2, 1),
                    lambda: vplus(1, [8, 9]),
                    lambda: vplus(1, [10, 11]),
                    lambda: proj_group("v", 3, 1),
                    lambda: vplus(1, [12, 13]),
                    lambda: vplus(1, [14, 15]),
                    lambda: proj_group("q", 2, 0),
                    lambda: proj_group("q", 2, 1),
                ],
                3: [lambda tt=tt: wo_piece(0, tt) for tt in range(4)],
                4: [
                    lambda: proj_group("q", 3, 0),
                    lambda: proj_group("q", 3, 1),
                ],
                5: [lambda tt=tt: wo_piece(1, tt) for tt in range(4)],
                7: [lambda tt=tt: wo_piece(2, tt) for tt in range(4)],
                9: [lambda tt=tt: wo_piece(3, tt) for tt in range(4)],
            }

            windows = [(c, t) for c in range(NIB) for t in range(NP)]
            av_st = None
            for idx in range(len(windows) + 2):
                w = windows[idx] if idx < len(windows) else None
                cur = {"c": w[0], "t": w[1], "e": {}} if w is not None else None
                items = bg.get(idx, [])
                done = 0
                for jc in range(JC):
                    want = (jc + 1) * len(items) // JC
                    while done < want:
                        items[done]()
                        done += 1
                    if cur is not None:
                        emit_scores(cur, jc)
                    if av_st is not None:
                        emit_av(av_st, jc)
                if av_st is not None:
                    emit_normalize(av_st)
                av_st = cur

    nc.compile()
    return nc


def build_in_maps(q, k, v, Wq, Wk, Wv, Wo):
    import ml_dtypes

    bf = ml_dtypes.bfloat16
    q = np.asarray(q, dtype=np.float32)
    k = np.asarray(k, dtype=np.float32)
    v = np.asarray(v, dtype=np.float32)
    Wq = np.asarray(Wq, dtype=np.float32)
    Wk = np.asarray(Wk, dtype=np.float32)
    Wv = np.asarray(Wv, dtype=np.float32)
    Wo = np.asarray(Wo, dtype=np.float32)

    in_maps = []
    for b in range(BATCH):
        qT = np.ascontiguousarray(q[b].T.astype(bf))
        kT = np.ascontiguousarray(k[b].T.astype(bf))
        vT = np.ascontiguousarray(v[b].T.astype(bf))
        for g in range(4):
            sl = slice(g * HG, (g + 1) * HG)
            in_maps.append(
                {
                    "xq": qT,
                    "xk": kT,
                    "xv": vT,
                    "wq": np.ascontiguousarray(Wq[sl, :].T.astype(bf)),
                    "wk": np.ascontiguousarray(Wk[sl, :].T.astype(bf)),
                    "wv": np.ascontiguousarray(Wv[sl, :].T.astype(bf)),
                    "wo": np.ascontiguousarray(Wo[:, sl].T),
                }
            )
    return in_maps


def combine_results(results):
    acc = []
    for b in range(BATCH):
        o = results[b * 4]["out"].astype(np.float32)
        for g in range(1, 4):
            o = o + results[b * 4 + g]["out"]
        acc.append(o)
    return np.stack(acc).reshape(BATCH, SEQ, D)


def kernel(q, k, v, Wq, Wk, Wv, Wo):
    in_maps = build_in_maps(q, k, v, Wq, Wk, Wv, Wo)
    nc = build_bass()

    def run_once():
        res = run_bass_kernel_spmd(nc, in_maps, core_ids=list(range(N_CORES)))
        return combine_results(res.results)

    acc = run_once()
    if not np.isfinite(acc).all():
        acc = run_once()  # guard against sporadic device flake
    return acc


# revision 22
# speedup vs baseline: 1.0448x; 1.0448x over previous
"""Trainium2 Bass kernel for 16-head MHA (B=2, S=2048, D=1024), fp32 I/O.

Sharding: 2-way batch-parallel x 4-way head-parallel across 8 NeuronCores.
Core c = (b, g) owns batch b and head group g (4 heads, a 256-wide slice of
the QKV projection output and of Wo's input dim). Each core computes its
(batch, head-group)'s full attention plus a partial output projection over
its batch's tokens; the host sums the 4 partials per batch. Versus pure
head-sharding this halves both input DMA (12MB/core) and output DMA (8MB).

Per-core dataflow (feature-major; host pre-transposes and casts to bf16):
  QT/KT [2x 128, 2048] bf16 = W_pair @ x.T    per head-pair tile
  V+ tiles [128j, 65] bf16                    VT j-chunks PE-transposed + ones
  windows = (i-block 512, head-pair t), t-major order, 8 windows:
    S.T pair [128j, 1024] = KT.T @ QT         both heads of the pair land in
                                              opposite PE row-halves and run
                                              CONCURRENTLY (row tiling)
    E = exp(S.T/8)                            one ACT instr per pair
    O+ [65, 512] += V+.T @ E_h                ones row = softmax denominator
    oc = O+[0:64] * replicate(1/O+[64])       DVE + PE outer-product replicate
    out rows = oc.T @ WoT  (fp32r)            partial projection
  K/V/Q projections and Wo are interleaved into the window loop as
  background work so the PE never idles; exp is the only ACT work and all
  DMA dispatch lives on SP (inputs, deadline-ordered) and DVE (outputs).

PSUM budget (8 banks): scores pair tiles 2x2 + AV accumulators 2 + shared
aux (proj/Wo/transpose/replicate) 2.
"""

import sys

sys.path.insert(0, "/opt/trn_rl_repo")

import numpy as np

import concourse.bacc as bacc
import concourse.mybir as mybir
import concourse.tile as tile
from concourse.bass_utils import run_bass_kernel_spmd
from concourse.masks import make_identity

F32 = mybir.dt.float32
R = mybir.dt.float32r
BF16 = mybir.dt.bfloat16
EXP = mybir.ActivationFunctionType.Exp

D = 1024
BATCH = 2
SEQ = 2048
DK = 64
NH = 4  # heads per core
HG = NH * DK  # 256-wide head-group slice per core
NP = 2  # head-pairs per core
N_CORES = 8
KT_TILES = D // 128  # 8 contraction tiles for the projections
NCH = SEQ // 512  # 4 token chunks
JC = SEQ // 128  # 16 j-chunks
IB = 512  # i-block (query window)
NIB = SEQ // IB  # 4
SCALE = 1.0 / np.sqrt(DK)


def build_bass():
    nc = bacc.Bacc(None)

    xq = nc.dram_tensor("xq", [D, SEQ], BF16, kind="ExternalInput")
    xk = nc.dram_tensor("xk", [D, SEQ], BF16, kind="ExternalInput")
    xv = nc.dram_tensor("xv", [D, SEQ], BF16, kind="ExternalInput")
    wq = nc.dram_tensor("wq", [D, HG], BF16, kind="ExternalInput")
    wk = nc.dram_tensor("wk", [D, HG], BF16, kind="ExternalInput")
    wv = nc.dram_tensor("wv", [D, HG], BF16, kind="ExternalInput")
    wo = nc.dram_tensor("wo", [HG, D], R, kind="ExternalInput")
    out = nc.dram_tensor("out", [SEQ, D], F32, kind="ExternalOutput")

    with tile.TileContext(nc) as tc:
        with (
            tc.tile_pool(name="consts", bufs=1) as cst,
            tc.tile_pool(name="wpool", bufs=1) as wpool,
            tc.tile_pool(name="acts", bufs=1) as acts,
            tc.tile_pool(name="vp", bufs=1) as vp_pool,
            tc.tile_pool(name="stage", bufs=1) as stage,
            tc.tile_pool(name="vtc", bufs=2) as vtc,
            tc.tile_pool(name="epool", bufs=20) as epool,
            tc.tile_pool(name="ocpool", bufs=1) as ocpool,
            tc.tile_pool(name="outpool", bufs=2) as outpool,
            tc.tile_pool(name="small", bufs=2) as small,
            tc.tile_pool(name="psb", bufs=2, space="PSUM") as psb,
            tc.tile_pool(name="pso", bufs=2, space="PSUM") as pso,
            tc.tile_pool(name="paux", bufs=2, space="PSUM") as paux,
        ):
            # --- constants ---
            ident_f = cst.tile([128, 128], F32)
            make_identity(nc, ident_f)
            ident = cst.tile([128, 128], R)
            nc.vector.tensor_copy(ident[:], ident_f[:])

            ones_f = cst.tile([128, 1], F32)
            nc.gpsimd.memset(ones_f[:], 1.0)
            onescol = cst.tile([128, 1], BF16)
            nc.vector.tensor_copy(onescol[:], ones_f[:])
            ones64 = cst.tile([1, 64], R)
            nc.vector.tensor_copy(ones64[:], ones_f[0:1, 0:1].to_broadcast([1, 64]))

            # warm the ACT exp table while DMA streams inputs
            scratch = cst.tile([1, 64], F32)
            nc.scalar.activation(
                scratch[:], ones_f[0:1, 0:1].to_broadcast([1, 64]), EXP
            )

            # --- weight tiles ---
            wq_sb = wpool.tile([128, KT_TILES, HG], BF16)
            wk_sb = wpool.tile([128, KT_TILES, HG], BF16)
            wv_sb = wpool.tile([128, KT_TILES, HG], BF16)
            wo_sb = [wpool.tile([128, D], R, name=f"wo{t}") for t in range(NP)]

            # --- activation tiles ---
            QT = [acts.tile([128, SEQ], BF16, name=f"QT{t}") for t in range(NP)]
            KT = [acts.tile([128, SEQ], BF16, name=f"KT{t}") for t in range(NP)]

            # --- input staging (chunks held until their pair-1 proj reads) ---
            xst = {}  # (tensor_key, chunk) -> tile
            x_r = {
                "q": xq.rearrange("(ko p) n -> p ko n", p=128),
                "k": xk.rearrange("(ko p) n -> p ko n", p=128),
                "v": xv.rearrange("(ko p) n -> p ko n", p=128),
            }

            def dma_chunk(key, c, bufs, eng=None):
                t = stage.tile(
                    [128, KT_TILES, 512], BF16, tag=f"x{key}", bufs=bufs,
                    name=f"x{key}{c}",
                )
                (eng or nc.sync).dma_start(
                    t[:], x_r[key][:, :, c * 512 : (c + 1) * 512]
                )
                xst[(key, c)] = t

            # Input DMA: K/Q stream on the SP HWDGE ring, V/Wo on the ACT
            # HWDGE ring (dispatched before the first exp, transfers run
            # concurrently with the K stream so window 0's K chunks are not
            # queued behind V). k/q chunks are consumed by both head-pairs'
            # projections adjacently (bufs=2); v chunks stay staged until the
            # deferred pair-1 V projection (bufs=4).
            nc.sync.dma_start(wk_sb[:], wk.rearrange("(ko p) n -> p ko n", p=128))
            dma_chunk("k", 0, 2)
            nc.sync.dma_start(wq_sb[:], wq.rearrange("(ko p) n -> p ko n", p=128))
            dma_chunk("q", 0, 2)
            nc.scalar.dma_start(
                wv_sb[:], wv.rearrange("(ko p) n -> p ko n", p=128)
            )
            dma_chunk("k", 1, 2)
            dma_chunk("v", 0, 4, eng=nc.scalar)
            dma_chunk("k", 2, 2)
            dma_chunk("v", 1, 4, eng=nc.scalar)
            dma_chunk("k", 3, 2)
            dma_chunk("q", 1, 2)
            dma_chunk("q", 2, 2)
            dma_chunk("q", 3, 2)
            dma_chunk("v", 2, 4)
            dma_chunk("v", 3, 4)
            for t in range(NP):
                nc.sync.dma_start(wo_sb[t][:], wo[t * 128 : (t + 1) * 128, :])

            w_sb = {"q": wq_sb, "k": wk_sb, "v": wv_sb}
            vp_tiles = {}  # (h, jg) -> V+ tile
            vt_chunks = {}  # (t, c) -> VT chunk tile (fp32r)

            def proj_group(key, c, t):
                """Project x[key] chunk c for head-pair t."""
                pp = paux.tile([128, 512], F32, tag="aux", name=f"pp_{key}{c}{t}")
                for k in range(KT_TILES):
                    nc.tensor.matmul(
                        pp[:],
                        w_sb[key][:, k, t * 128 : (t + 1) * 128],
                        xst[(key, c)][:, k, :],
                        start=(k == 0),
                        stop=(k == KT_TILES - 1),
                    )
                if key == "v":
                    vt = vtc.tile([128, 512], R, tag="vt", name=f"vt{t}{c}")
                    nc.vector.tensor_copy(vt[:], pp[:])
                    vt_chunks[(t, c)] = vt
                else:
                    TT = QT[t] if key == "q" else KT[t]
                    nc.vector.tensor_copy(TT[:, c * 512 : (c + 1) * 512], pp[:])

            def vplus(t, jgs):
                """Build V+ tiles for both heads of pair t, j-groups jgs."""
                for jg in jgs:
                    vt = vt_chunks[(t, jg // 4)]
                    jl = (jg % 4) * 128
                    for r in range(2):
                        h = 2 * t + r
                        hs = slice(r * 64, (r + 1) * 64)
                        tp = paux.tile([128, 64], R, tag="aux", name=f"tp{h}{jg}")
                        nc.tensor.transpose(
                            tp[:], vt[hs, jl : jl + 128], ident[hs, hs]
                        )
                        vpt = vp_pool.tile(
                            [128, DK + 1], BF16, tag=f"vp_{h}_{jg}", name=f"vp{h}{jg}"
                        )
                        nc.vector.tensor_copy(vpt[:, 0:DK], tp[:])
                        nc.vector.tensor_copy(vpt[:, DK : DK + 1], onescol[:])
                        vp_tiles[(h, jg)] = vpt

            ocs = {}  # (c, t) -> oc tile [128, 512] fp32r

            def norm_a(st):
                """At the window boundary: evacuate the AV accumulators from
                PSUM so the po banks free up for the next window's AV almost
                immediately. The reciprocal chain runs later as bg work."""
                c, t, po = st["c"], st["t"], st["po"]
                st["oc"] = ocpool.tile(
                    [128, IB], R, tag=f"oc{t}", bufs=2, name=f"oc{c}{t}"
                )
                st["posb"] = {}
                for r in range(2):
                    pb = small.tile(
                        [DK + 1, IB], F32, tag="posb", bufs=4, name=f"pb{r}"
                    )
                    nc.vector.tensor_copy(pb[:], po[r][:])
                    st["posb"][r] = pb
                ocs[(c, t)] = st["oc"]

            def norm_b(st, r):
                """Per-head softmax normalization from the SBUF copy."""
                oc, pb = st["oc"], st["posb"][r]
                rrow = small.tile([1, IB], F32, tag="rrow", name=f"rr{r}")
                nc.vector.reciprocal(rrow[:], pb[DK : DK + 1, :])
                rcr = small.tile([1, IB], R, tag="rcr", name=f"rcr{r}")
                nc.vector.tensor_copy(rcr[:], rrow[:])
                rep = paux.tile([64, IB], F32, tag="aux", name=f"rep{r}")
                nc.tensor.matmul(rep[:], ones64[:], rcr[:], start=True, stop=True)
                rec_sb = small.tile([64, IB], F32, tag="recsb", name=f"rs{r}")
                nc.vector.tensor_copy(rec_sb[:], rep[:])
                nc.vector.tensor_tensor(
                    oc[r * 64 : (r + 1) * 64, :],
                    pb[0:DK, :],
                    rec_sb[:],
                    mybir.AluOpType.mult,
                )

            def wo_piece(c, tt):
                """Output projection for token tile tt of i-block c."""
                os_t = outpool.tile([128, D], F32, tag="os", name=f"os{c}{tt}")
                for oh in range(2):
                    wo_ps = paux.tile(
                        [128, 512], F32, tag="aux", name=f"wops{c}{tt}{oh}"
                    )
                    for t in range(NP):
                        nc.tensor.matmul(
                            wo_ps[:],
                            ocs[(c, t)][:, tt * 128 : (tt + 1) * 128],
                            wo_sb[t][:, oh * 512 : (oh + 1) * 512],
                            start=(t == 0),
                            stop=(t == NP - 1),
                        )
                    nc.vector.tensor_copy(
                        os_t[:, oh * 512 : (oh + 1) * 512], wo_ps[:]
                    )
                r0 = c * IB + tt * 128
                nc.sync.dma_start(out[r0 : r0 + 128, :], os_t[:])

            def emit_scores(st, jc):
                c, t = st["c"], st["t"]
                i0 = c * IB
                ps = psb.tile([128, 2 * IB], F32, tag="ps", name=f"ps{jc}")
                for r in range(2):
                    hs = slice(r * 64, (r + 1) * 64)
                    nc.tensor.matmul(
                        ps[:, r * IB : (r + 1) * IB],
                        KT[t][hs, jc * 128 : (jc + 1) * 128],
                        QT[t][hs, i0 : i0 + IB],
                        start=True,
                        stop=True,
                    )
                e_t = epool.tile([128, 2 * IB], BF16, tag="e", name=f"e{jc}")
                nc.scalar.activation(e_t[:], ps[:], EXP, scale=SCALE)
                st["e"][jc] = e_t

            def emit_av(st, jc):
                t = st["t"]
                if jc == 0:
                    st["po"] = {
                        r: pso.tile([DK + 1, IB], F32, tag="po", name=f"po{r}")
                        for r in range(2)
                    }
                po = st["po"]
                for r in range(2):
                    nc.tensor.matmul(
                        po[r][:],
                        vp_tiles[(2 * t + r, jc)][:],
                        st["e"][jc][:, r * IB : (r + 1) * IB],
                        start=(jc == 0),
                        stop=(jc == JC - 1),
                    )

            # --- prefix: chunk-0 K/Q for both pairs ---
            proj_group("k", 0, 0)
            proj_group("k", 0, 1)
            proj_group("q", 0, 0)
            proj_group("q", 0, 1)

            # --- background work plan (c-major window order) ---
            bg = {
                0: [
                    lambda: proj_group("k", 1, 0),
                    lambda: proj_group("k", 1, 1),
                    lambda: proj_group("k", 2, 0),
                    lambda: proj_group("k", 2, 1),
                    lambda: proj_group("k", 3, 0),
                    lambda: proj_group("k", 3, 1),
                    lambda: proj_group("v", 0, 0),
                    lambda: vplus(0, [0, 1]),
                    lambda: vplus(0, [2, 3]),
                    lambda: proj_group("v", 1, 0),
                    lambda: vplus(0, [4, 5]),
                ],
                1: [
                    lambda: vplus(0, [6, 7]),
                    lambda: proj_group("v", 2, 0),
                    lambda: vplus(0, [8, 9]),
                    lambda: vplus(0, [10, 11]),
                    lambda: proj_group("q", 1, 0),
                    lambda: proj_group("v", 3, 0),
                    lambda: vplus(0, [12, 13]),
                    lambda: proj_group("q", 1, 1),
                    lambda: vplus(0, [14, 15]),
                    lambda: proj_group("v", 0, 1),
                    lambda: vplus(1, [0, 1]),
                    lambda: vplus(1, [2, 3]),
                ],
                2: [
                    lambda: proj_group("v", 1, 1),
                    lambda: vplus(1, [4, 5]),
                    lambda: vplus(1, [6, 7]),
                    lambda: proj_group("v", 2, 1),
                    lambda: vplus(1, [8, 9]),
                    lambda: vplus(1, [10, 11]),
                    lambda: proj_group("v", 3, 1),
                    lambda: vplus(1, [12, 13]),
                    lambda: vplus(1, [14, 15]),
                    lambda: proj_group("q", 2, 0),
                    lambda: proj_group("q", 2, 1),
                ],
                3: [lambda tt=tt: wo_piece(0, tt) for tt in range(4)],
                4: [
                    lambda: proj_group("q", 3, 0),
                    lambda: proj_group("q", 3, 1),
                ],
                5: [lambda tt=tt: wo_piece(1, tt) for tt in range(4)],
                7: [lambda tt=tt: wo_piece(2, tt) for tt in range(4)],
                9: [lambda tt=tt: wo_piece(3, tt) for tt in range(4)],
            }

            windows = [(c, t) for c in range(NIB) for t in range(NP)]
            av_st = None
            norm_st = None  # window whose reciprocal chain is pending
            for idx in range(len(windows) + 2):
                w = windows[idx] if idx < len(windows) else None
                cur = {"c": w[0], "t": w[1], "e": {}} if w is not None else None
                items = list(bg.get(idx, []))
                if norm_st is not None:
                    st = norm_st
                    items = [
                        lambda: norm_b(st, 0),
                        lambda: norm_b(st, 1),
                    ] + items
                done = 0
                for jc in range(JC):
                    want = (jc + 1) * len(items) // JC
                    while done < want:
                        items[done]()
                        done += 1
                    if cur is not None:
                        emit_scores(cur, jc)
                    if av_st is not None:
                        emit_av(av_st, jc)
                if av_st is not None:
                    norm_a(av_st)
                norm_st = av_st
                av_st = cur

    nc.compile()
    return nc


def build_in_maps(q, k, v, Wq, Wk, Wv, Wo):
    import ml_dtypes

    bf = ml_dtypes.bfloat16
    q = np.asarray(q, dtype=np.float32)
    k = np.asarray(k, dtype=np.float32)
    v = np.asarray(v, dtype=np.float32)
    Wq = np.asarray(Wq, dtype=np.float32)
    Wk = np.asarray(Wk, dtype=np.float32)
    Wv = np.asarray(Wv, dtype=np.float32)
    Wo = np.asarray(Wo, dtype=np.float32)

    in_maps = []
    for b in range(BATCH):
        qT = np.ascontiguousarray(q[b].T.astype(bf))
        kT = np.ascontiguousarray(k[b].T.astype(bf))
        vT = np.ascontiguousarray(v[b].T.astype(bf))
        for g in range(4):
            sl = slice(g * HG, (g + 1) * HG)
            in_maps.append(
                {
                    "xq": qT,
                    "xk": kT,
                    "xv": vT,
                    "wq": np.ascontiguousarray(Wq[sl, :].T.astype(bf)),
                    "wk": np.ascontiguousarray(Wk[sl, :].T.astype(bf)),
                    "wv": np.ascontiguousarray(Wv[sl, :].T.astype(bf)),
                    "wo": np.ascontiguousarray(Wo[:, sl].T),
                }
            )
    return in_maps


def combine_results(results):
    acc = []
    for b in range(BATCH):
        o = results[b * 4]["out"].astype(np.float32)
        for g in range(1, 4):
            o = o + results[b * 4 + g]["out"]
        acc.append(o)
    return np.stack(acc).reshape(BATCH, SEQ, D)


def kernel(q, k, v, Wq, Wk, Wv, Wo):
    in_maps = build_in_maps(q, k, v, Wq, Wk, Wv, Wo)
    nc = build_bass()

    def run_once():
        res = run_bass_kernel_spmd(nc, in_maps, core_ids=list(range(N_CORES)))
        return combine_results(res.results)

    acc = run_once()
    if not np.isfinite(acc).all():
        acc = run_once()  # guard against sporadic device flake
    return acc


# revision 29
# speedup vs baseline: 1.0704x; 1.0246x over previous
"""Trainium2 Bass kernel for 16-head MHA (B=2, S=2048, D=1024), fp32 I/O.

Sharding: 2-way batch-parallel x 4-way head-parallel across 8 NeuronCores.
Core c = (b, g) owns batch b and head group g (4 heads, a 256-wide slice of
the QKV projection output and of Wo's input dim). Each core computes its
(batch, head-group)'s full attention plus a partial output projection over
its batch's tokens; the host sums the 4 partials per batch. Versus pure
head-sharding this halves both input DMA (12MB/core) and output DMA (8MB).

Per-core dataflow (feature-major; host pre-transposes and casts to bf16):
  QT/KT [2x 128, 2048] bf16 = W_pair @ x.T    per head-pair tile
  V+ tiles [128j, 65] bf16                    VT j-chunks PE-transposed + ones
  windows = (i-block 512, head-pair t), t-major order, 8 windows:
    S.T pair [128j, 1024] = KT.T @ QT         both heads of the pair land in
                                              opposite PE row-halves and run
                                              CONCURRENTLY (row tiling)
    E = exp(S.T/8)                            one ACT instr per pair
    O+ [65, 512] += V+.T @ E_h                ones row = softmax denominator
    oc = O+[0:64] * replicate(1/O+[64])       DVE + PE outer-product replicate
    out rows = oc.T @ WoT  (fp32r)            partial projection
  K/V/Q projections and Wo are interleaved into the window loop as
  background work so the PE never idles; exp is the only ACT work and all
  DMA dispatch lives on SP (inputs, deadline-ordered) and DVE (outputs).

PSUM budget (8 banks): scores pair tiles 2x2 + AV accumulators 2 + shared
aux (proj/Wo/transpose/replicate) 2.
"""

import sys

sys.path.insert(0, "/opt/trn_rl_repo")

import numpy as np

import concourse.bacc as bacc
import concourse.mybir as mybir
import concourse.tile as tile
from concourse.bass_utils import run_bass_kernel_spmd
from concourse.masks import make_identity

F32 = mybir.dt.float32
R = mybir.dt.float32r
BF16 = mybir.dt.bfloat16
EXP = mybir.ActivationFunctionType.Exp

D = 1024
BATCH = 2
SEQ = 2048
DK = 64
NH = 4  # heads per core
HG = NH * DK  # 256-wide head-group slice per core
NP = 2  # head-pairs per core
N_CORES = 8
KT_TILES = D // 128  # 8 contraction tiles for the projections
NCH = SEQ // 512  # 4 token chunks
JC = SEQ // 128  # 16 j-chunks
IB = 512  # i-block (query window)
NIB = SEQ // IB  # 4
SCALE = 1.0 / np.sqrt(DK)


def build_bass():
    nc = bacc.Bacc(None)

    xq = nc.dram_tensor("xq", [D, SEQ], BF16, kind="ExternalInput")
    xk = nc.dram_tensor("xk", [D, SEQ], BF16, kind="ExternalInput")
    xv = nc.dram_tensor("xv", [D, SEQ], BF16, kind="ExternalInput")
    wq = nc.dram_tensor("wq", [D, HG], BF16, kind="ExternalInput")
    wk = nc.dram_tensor("wk", [D, HG], BF16, kind="ExternalInput")
    wv = nc.dram_tensor("wv", [D, HG], BF16, kind="ExternalInput")
    wo = nc.dram_tensor("wo", [HG, D], R, kind="ExternalInput")
    out = nc.dram_tensor("out", [SEQ, D], F32, kind="ExternalOutput")

    with tile.TileContext(nc) as tc:
        with (
            tc.tile_pool(name="consts", bufs=1) as cst,
            tc.tile_pool(name="wpool", bufs=1) as wpool,
            tc.tile_pool(name="acts", bufs=1) as acts,
            tc.tile_pool(name="vp", bufs=1) as vp_pool,
            tc.tile_pool(name="stage", bufs=1) as stage,
            tc.tile_pool(name="vtc", bufs=2) as vtc,
            tc.tile_pool(name="epool", bufs=20) as epool,
            tc.tile_pool(name="ocpool", bufs=1) as ocpool,
            tc.tile_pool(name="outpool", bufs=2) as outpool,
            tc.tile_pool(name="small", bufs=2) as small,
            tc.tile_pool(name="psb", bufs=2, space="PSUM") as psb,
            tc.tile_pool(name="pso", bufs=2, space="PSUM") as pso,
            tc.tile_pool(name="paux", bufs=2, space="PSUM") as paux,
        ):
            # --- constants ---
            ident_f = cst.tile([128, 128], F32)
            make_identity(nc, ident_f)
            ident = cst.tile([128, 128], R)
            nc.vector.tensor_copy(ident[:], ident_f[:])

            ones_f = cst.tile([128, 1], F32)
            nc.gpsimd.memset(ones_f[:], 1.0)
            onescol = cst.tile([128, 1], BF16)
            nc.vector.tensor_copy(onescol[:], ones_f[:])
            ones64 = cst.tile([1, 64], R)
            nc.vector.tensor_copy(ones64[:], ones_f[0:1, 0:1].to_broadcast([1, 64]))

            # warm the ACT exp table while DMA streams inputs
            scratch = cst.tile([1, 64], F32)
            nc.scalar.activation(
                scratch[:], ones_f[0:1, 0:1].to_broadcast([1, 64]), EXP
            )

            # spin the PE while the first input chunks stream in: ~40 dummy
            # matmuls keep the HAM activity window busy so the clock is at
            # K=8/8 (2.4 GHz) when the real projections start
            ident_bf = cst.tile([128, 128], BF16)
            nc.vector.tensor_copy(ident_bf[:], ident_f[:])
            warm_in = cst.tile([128, 512], BF16)
            nc.gpsimd.memset(warm_in[:], 0.125)
            for _ in range(40):
                wps = paux.tile([128, 512], F32, tag="aux", name="warm")
                nc.tensor.matmul(
                    wps[:], ident_bf[:], warm_in[:], start=True, stop=True
                )

            # --- weight tiles ---
            wq_sb = wpool.tile([128, KT_TILES, HG], BF16)
            wk_sb = wpool.tile([128, KT_TILES, HG], BF16)
            wv_sb = wpool.tile([128, KT_TILES, HG], BF16)
            wo_sb = [wpool.tile([128, D], R, name=f"wo{t}") for t in range(NP)]

            # --- activation tiles ---
            QT = [acts.tile([128, SEQ], BF16, name=f"QT{t}") for t in range(NP)]
            KT = [acts.tile([128, SEQ], BF16, name=f"KT{t}") for t in range(NP)]

            # --- input staging (chunks held until their pair-1 proj reads) ---
            xst = {}  # (tensor_key, chunk) -> tile
            x_r = {
                "q": xq.rearrange("(ko p) n -> p ko n", p=128),
                "k": xk.rearrange("(ko p) n -> p ko n", p=128),
                "v": xv.rearrange("(ko p) n -> p ko n", p=128),
            }

            def dma_chunk(key, c, bufs, eng=None):
                t = stage.tile(
                    [128, KT_TILES, 512], BF16, tag=f"x{key}", bufs=bufs,
                    name=f"x{key}{c}",
                )
                (eng or nc.sync).dma_start(
                    t[:], x_r[key][:, :, c * 512 : (c + 1) * 512]
                )
                xst[(key, c)] = t

            # Input DMA split across BOTH HWDGE rings (SP + ACT) in deadline
            # order — a single ring moves ~1MB per 4-5us, too slow for the
            # first window's K/Q needs. All ACT-ring dispatches are emitted
            # before the first exp. k/q chunks are consumed by both
            # head-pairs' projections adjacently (bufs=2); v chunks stay
            # staged until the deferred pair-1 V projection (bufs=4).
            nc.sync.dma_start(wk_sb[:], wk.rearrange("(ko p) n -> p ko n", p=128))
            dma_chunk("k", 0, 2)
            nc.scalar.dma_start(
                wq_sb[:], wq.rearrange("(ko p) n -> p ko n", p=128)
            )
            dma_chunk("q", 0, 2, eng=nc.scalar)
            dma_chunk("k", 1, 2)
            dma_chunk("k", 2, 2, eng=nc.scalar)
            dma_chunk("k", 3, 2)
            nc.scalar.dma_start(
                wv_sb[:], wv.rearrange("(ko p) n -> p ko n", p=128)
            )
            dma_chunk("q", 1, 2, eng=nc.scalar)
            dma_chunk("v", 0, 4)
            dma_chunk("v", 1, 4, eng=nc.scalar)
            dma_chunk("q", 2, 2)
            dma_chunk("v", 2, 4, eng=nc.scalar)
            dma_chunk("v", 3, 4)
            dma_chunk("q", 3, 2, eng=nc.scalar)
            for t in range(NP):
                nc.sync.dma_start(wo_sb[t][:], wo[t * 128 : (t + 1) * 128, :])

            w_sb = {"q": wq_sb, "k": wk_sb, "v": wv_sb}
            vp_tiles = {}  # (h, jg) -> V+ tile
            vt_chunks = {}  # (t, c) -> VT chunk tile (fp32r)

            def proj_group(key, c, t):
                """Project x[key] chunk c for head-pair t."""
                pp = paux.tile([128, 512], F32, tag="aux", name=f"pp_{key}{c}{t}")
                for k in range(KT_TILES):
                    nc.tensor.matmul(
                        pp[:],
                        w_sb[key][:, k, t * 128 : (t + 1) * 128],
                        xst[(key, c)][:, k, :],
                        start=(k == 0),
                        stop=(k == KT_TILES - 1),
                    )
                if key == "v":
                    vt = vtc.tile([128, 512], R, tag="vt", name=f"vt{t}{c}")
                    nc.vector.tensor_copy(vt[:], pp[:])
                    vt_chunks[(t, c)] = vt
                else:
                    TT = QT[t] if key == "q" else KT[t]
                    nc.vector.tensor_copy(TT[:, c * 512 : (c + 1) * 512], pp[:])

            def vplus(t, jgs):
                """Build V+ tiles for both heads of pair t, j-groups jgs."""
                for jg in jgs:
                    vt = vt_chunks[(t, jg // 4)]
                    jl = (jg % 4) * 128
                    for r in range(2):
                        h = 2 * t + r
                        hs = slice(r * 64, (r + 1) * 64)
                        tp = paux.tile([128, 64], R, tag="aux", name=f"tp{h}{jg}")
                        nc.tensor.transpose(
                            tp[:], vt[hs, jl : jl + 128], ident[hs, hs]
                        )
                        vpt = vp_pool.tile(
                            [128, DK + 1], BF16, tag=f"vp_{h}_{jg}", name=f"vp{h}{jg}"
                        )
                        nc.vector.tensor_copy(vpt[:, 0:DK], tp[:])
                        nc.vector.tensor_copy(vpt[:, DK : DK + 1], onescol[:])
                        vp_tiles[(h, jg)] = vpt

            ocs = {}  # (c, t) -> oc tile [128, 512] fp32r

            def norm_a(st):
                """At the window boundary: evacuate the AV accumulators from
                PSUM so the po banks free up for the next window's AV almost
                immediately. The reciprocal chain runs later as bg work."""
                c, t, po = st["c"], st["t"], st["po"]
                st["oc"] = ocpool.tile(
                    [128, IB], R, tag=f"oc{t}", bufs=2, name=f"oc{c}{t}"
                )
                st["posb"] = {}
                for r in range(2):
                    pb = small.tile(
                        [DK + 1, IB], F32, tag="posb", bufs=4, name=f"pb{r}"
                    )
                    nc.vector.tensor_copy(pb[:], po[r][:])
                    st["posb"][r] = pb
                ocs[(c, t)] = st["oc"]

            def norm_b1(st):
                """DVE-only part of the normalization (slow reciprocal) —
                emitted at window start so it never blocks the PE queue."""
                st["rcr"] = {}
                for r in range(2):
                    pb = st["posb"][r]
                    rrow = small.tile([1, IB], F32, tag="rrow", name=f"rr{r}")
                    nc.vector.reciprocal(rrow[:], pb[DK : DK + 1, :])
                    rcr = small.tile([1, IB], R, tag="rcr", name=f"rcr{r}")
                    nc.vector.tensor_copy(rcr[:], rrow[:])
                    st["rcr"][r] = rcr

            def norm_b2(st, r):
                """PE replicate + final multiply, emitted mid-window once the
                reciprocal row is long done."""
                oc, pb = st["oc"], st["posb"][r]
                rep = paux.tile([64, IB], F32, tag="aux", name=f"rep{r}")
                nc.tensor.matmul(
                    rep[:], ones64[:], st["rcr"][r][:], start=True, stop=True
                )
                rec_sb = small.tile([64, IB], F32, tag="recsb", name=f"rs{r}")
                nc.vector.tensor_copy(rec_sb[:], rep[:])
                nc.vector.tensor_tensor(
                    oc[r * 64 : (r + 1) * 64, :],
                    pb[0:DK, :],
                    rec_sb[:],
                    mybir.AluOpType.mult,
                )

            def wo_piece(c, tt):
                """Output projection for token tile tt of i-block c."""
                os_t = outpool.tile([128, D], F32, tag="os", name=f"os{c}{tt}")
                for oh in range(2):
                    wo_ps = paux.tile(
                        [128, 512], F32, tag="aux", name=f"wops{c}{tt}{oh}"
                    )
                    for t in range(NP):
                        nc.tensor.matmul(
                            wo_ps[:],
                            ocs[(c, t)][:, tt * 128 : (tt + 1) * 128],
                            wo_sb[t][:, oh * 512 : (oh + 1) * 512],
                            start=(t == 0),
                            stop=(t == NP - 1),
                        )
                    nc.vector.tensor_copy(
                        os_t[:, oh * 512 : (oh + 1) * 512], wo_ps[:]
                    )
                r0 = c * IB + tt * 128
                nc.sync.dma_start(out[r0 : r0 + 128, :], os_t[:])

            def emit_scores(st, jc):
                c, t = st["c"], st["t"]
                i0 = c * IB
                ps = psb.tile([128, 2 * IB], F32, tag="ps", name=f"ps{jc}")
                for r in range(2):
                    hs = slice(r * 64, (r + 1) * 64)
                    nc.tensor.matmul(
                        ps[:, r * IB : (r + 1) * IB],
                        KT[t][hs, jc * 128 : (jc + 1) * 128],
                        QT[t][hs, i0 : i0 + IB],
                        start=True,
                        stop=True,
                    )
                e_t = epool.tile([128, 2 * IB], BF16, tag="e", name=f"e{jc}")
                nc.scalar.activation(e_t[:], ps[:], EXP, scale=SCALE)
                st["e"][jc] = e_t

            def emit_av(st, jc):
                t = st["t"]
                if jc == 0:
                    st["po"] = {
                        r: pso.tile([DK + 1, IB], F32, tag="po", name=f"po{r}")
                        for r in range(2)
                    }
                po = st["po"]
                for r in range(2):
                    nc.tensor.matmul(
                        po[r][:],
                        vp_tiles[(2 * t + r, jc)][:],
                        st["e"][jc][:, r * IB : (r + 1) * IB],
                        start=(jc == 0),
                        stop=(jc == JC - 1),
                    )

            # --- prefix: chunk-0 K/Q for both pairs ---
            proj_group("k", 0, 0)
            proj_group("k", 0, 1)
            proj_group("q", 0, 0)
            proj_group("q", 0, 1)

            # --- background work plan (c-major window order) ---
            # Items are (due_slot, fn): the item is emitted no later than just
            # before slot `due_slot` of its window (None = purely paced).
            bg = {
                0: [
                    (4, lambda: proj_group("k", 1, 0)),
                    (None, lambda: proj_group("k", 1, 1)),
                    (8, lambda: proj_group("k", 2, 0)),
                    (None, lambda: proj_group("k", 2, 1)),
                    (12, lambda: proj_group("k", 3, 0)),
                    (None, lambda: proj_group("k", 3, 1)),
                    (13, lambda: proj_group("v", 0, 0)),
                    (14, lambda: vplus(0, [0, 1])),
                    (15, lambda: vplus(0, [2, 3])),
                    (None, lambda: proj_group("v", 1, 0)),
                    (None, lambda: vplus(0, [4, 5])),
                ],
                1: [
                    (6, lambda: vplus(0, [6, 7])),
                    (6, lambda: proj_group("v", 2, 0)),
                    (8, lambda: vplus(0, [8, 9])),
                    (10, lambda: vplus(0, [10, 11])),
                    (None, lambda: proj_group("q", 1, 0)),
                    (11, lambda: proj_group("v", 3, 0)),
                    (12, lambda: vplus(0, [12, 13])),
                    (None, lambda: proj_group("q", 1, 1)),
                    (14, lambda: vplus(0, [14, 15])),
                    (None, lambda: proj_group("v", 0, 1)),
                    (None, lambda: vplus(1, [0, 1])),
                    (None, lambda: vplus(1, [2, 3])),
                ],
                2: [
                    (2, lambda: proj_group("v", 1, 1)),
                    (4, lambda: vplus(1, [4, 5])),
                    (6, lambda: vplus(1, [6, 7])),
                    (6, lambda: proj_group("v", 2, 1)),
                    (8, lambda: vplus(1, [8, 9])),
                    (10, lambda: vplus(1, [10, 11])),
                    (10, lambda: proj_group("v", 3, 1)),
                    (12, lambda: vplus(1, [12, 13])),
                    (14, lambda: vplus(1, [14, 15])),
                    (None, lambda: proj_group("q", 2, 0)),
                    (None, lambda: proj_group("q", 2, 1)),
                ],
                3: [(None, lambda tt=tt: wo_piece(0, tt)) for tt in range(4)],
                4: [
                    (None, lambda: proj_group("q", 3, 0)),
                    (None, lambda: proj_group("q", 3, 1)),
                ],
                5: [(None, lambda tt=tt: wo_piece(1, tt)) for tt in range(4)],
                7: [(None, lambda tt=tt: wo_piece(2, tt)) for tt in range(4)],
                9: [(None, lambda tt=tt: wo_piece(3, tt)) for tt in range(4)],
            }

            windows = [(c, t) for c in range(NIB) for t in range(NP)]
            av_st = None
            norm_st = None  # window whose reciprocal chain is pending
            for idx in range(len(windows) + 2):
                w = windows[idx] if idx < len(windows) else None
                cur = {"c": w[0], "t": w[1], "e": {}} if w is not None else None
                base = list(bg.get(idx, []))
                if norm_st is not None:
                    st = norm_st
                    pre = [(None, lambda: norm_b1(st))]
                    mid = [
                        (None, lambda: norm_b2(st, 0)),
                        (None, lambda: norm_b2(st, 1)),
                    ]
                else:
                    pre, mid = [], []
                if idx in (3, 5, 7, 9):
                    # Wo pieces depend on the norm_b2 outputs; keep them after
                    items = pre + mid + base
                else:
                    items = pre + base[:2] + mid + base[2:]
                done = 0
                for jc in range(JC):
                    want = (jc + 1) * len(items) // JC
                    while done < len(items) and (
                        done < want
                        or any(
                            d is not None and d <= jc
                            for d, _ in items[done:]
                        )
                    ):
                        items[done][1]()
                        done += 1
                    if cur is not None:
                        emit_scores(cur, jc)
                    if av_st is not None:
                        emit_av(av_st, jc)
                while done < len(items):
                    items[done][1]()
                    done += 1
                if av_st is not None:
                    norm_a(av_st)
                norm_st = av_st
                av_st = cur

    nc.compile()
    return nc


def build_in_maps(q, k, v, Wq, Wk, Wv, Wo):
    import ml_dtypes

    bf = ml_dtypes.bfloat16
    q = np.asarray(q, dtype=np.float32)
    k = np.asarray(k, dtype=np.float32)
    v = np.asarray(v, dtype=np.float32)
    Wq = np.asarray(Wq, dtype=np.float32)
    Wk = np.asarray(Wk, dtype=np.float32)
    Wv = np.asarray(Wv, dtype=np.float32)
    Wo = np.asarray(Wo, dtype=np.float32)

    in_maps = []
    for b in range(BATCH):
        qT = np.ascontiguousarray(q[b].T.astype(bf))
        kT = np.ascontiguousarray(k[b].T.astype(bf))
        vT = np.ascontiguousarray(v[b].T.astype(bf))
        for g in range(4):
            sl = slice(g * HG, (g + 1) * HG)
            in_maps.append(
                {
                    "xq": qT,
                    "xk": kT,
                    "xv": vT,
                    "wq": np.ascontiguousarray(Wq[sl, :].T.astype(bf)),
                    "wk": np.ascontiguousarray(Wk[sl, :].T.astype(bf)),
                    "wv": np.ascontiguousarray(Wv[sl, :].T.astype(bf)),
                    "wo": np.ascontiguousarray(Wo[:, sl].T),
                }
            )
    return in_maps


def combine_results(results):
    acc = []
    for b in range(BATCH):
        o = results[b * 4]["out"].astype(np.float32)
        for g in range(1, 4):
            o = o + results[b * 4 + g]["out"]
        acc.append(o)
    return np.stack(acc).reshape(BATCH, SEQ, D)


def kernel(q, k, v, Wq, Wk, Wv, Wo):
    in_maps = build_in_maps(q, k, v, Wq, Wk, Wv, Wo)
    nc = build_bass()

    def run_once():
        res = run_bass_kernel_spmd(nc, in_maps, core_ids=list(range(N_CORES)))
        return combine_results(res.results)

    acc = run_once()
    if not np.isfinite(acc).all():
        acc = run_once()  # guard against sporadic device flake
    return acc
